# revision 1
# baseline (speedup 1.0000x reference)
"""Trainium2 Bass kernel for nn_MultiHeadAttention_56118042690041.

8-core sharding: batch x heads tensor-parallel.
  core c (0..7): batch b = c//4, heads 4*(c%4) .. 4*(c%4)+4 (as 2 packed pairs).
Per core (all matmul operands bf16, PSUM accumulation f32):
  - QKV projections. k/v: pair-packed stationary W, moving xT -> k2T/v2T in
    [dk, s] layout. q: flipped orientation (stationary xT chunk, moving W of
    all 4 heads) -> q2 directly in [t, dk] layout, assembled into
    ones-augmented per-head q2aug tiles (no PE transposes needed).
  - Attention per head (note reference's faithful "bug": scores = v2 @ k2^T,
    softmax, weighted sum of q2): software-pipelined blocks of (head, s-half):
    scores block j interleaved tb-wise with the AV matmuls of block j-1 so the
    PE never waits on exp. exp is split between ACT (exact, bf16 out) and DVE
    (Schraudolph bit-trick: round(A*x+B) -> int16, bitcast bf16; softmax-ratio
    errors cancel in the weighted average). AV uses a ones-column augmented
    q2aug so the softmax denominator falls out of the same matmul;
    normalization = reciprocal_approx_fast on the denominator row + PE
    broadcast + one DVE multiply; bias added on ACT.
  - Head outputs (bf16, [dk, s] layout) AllGather'd per pair across the 4
    cores of the batch group, overlapped with remaining attention work.
  - Output projection: each core computes a disjoint 256-wide d-slice of
    out = headout @ Wo^T + bo (column-sharded Wo), pair-0 contraction chunks
    first so matmuls start as soon as the first AllGather lands.
Host: slices/packs weights per core (bf16), transposes x (bf16), concatenates
disjoint outputs.
"""

import contextlib
import ctypes
import os
import sys
import types

import numpy as np

if "/opt/trn_rl_repo" not in sys.path:
    sys.path.insert(0, "/opt/trn_rl_repo")

# ---------------------------------------------------------------- shims ----


def _install_antenv_shim():
    """Provide antenv.axon_hooks (NTFF profile hook) if the image lacks it."""
    try:
        import antenv.axon_hooks  # noqa: F401

        return
    except ImportError:
        pass

    def _hook_factory():
        so_path = "/opt/axon/libaxon_pjrt.so"
        try:
            lib = ctypes.CDLL(so_path)
        except OSError:
            return None
        if not hasattr(lib, "axon_start_nrt_profile"):
            return None
        lib.axon_start_nrt_profile.argtypes = [
            ctypes.POINTER(ctypes.c_int64),
            ctypes.c_size_t,
        ]
        lib.axon_start_nrt_profile.restype = ctypes.c_int64
        lib.axon_stop_nrt_profile.argtypes = [ctypes.c_char_p]
        lib.axon_stop_nrt_profile.restype = ctypes.c_int64

        @contextlib.contextmanager
        def _hook(output_dir, device_ids):
            import jax

            jax.devices()
            if device_ids:
                ids = (ctypes.c_int64 * len(device_ids))(*device_ids)
                rc = lib.axon_start_nrt_profile(ids, len(device_ids))
            else:
                rc = lib.axon_start_nrt_profile(None, 0)
            if rc != 0:
                raise RuntimeError(f"axon_start_nrt_profile rc={rc}")
            try:
                yield
            finally:
                n = lib.axon_stop_nrt_profile(str(output_dir).encode())
                print(f"ntff profile: {n} file(s) -> {output_dir}", file=sys.stderr)

        return _hook

    hook = _hook_factory()
    mod = types.ModuleType("antenv.axon_hooks")
    mod.get_axon_ntff_profile_hook = lambda: hook
    mod.set_axon_ntff_profile_hook = lambda h: None
    sys.modules["antenv.axon_hooks"] = mod


def _install_tile_drain_patch():
    """This walrus build rejects >1 sync wait on the Tile tail Drain; split the
    waits across chained single-wait drains."""
    import concourse.tile as tile

    if getattr(tile.TileContext, "_drain_patch_installed", False):
        return

    def _drain_and_barrier(self, tick_clock, wait_clock):
        nc = self.nc
        drain_inst = nc.sync.drain()
        wait_clock.add_sem_waits(
            drain_inst.ins, tile.ScopedClock({None: tick_clock.global_clock})
        )
        si = drain_inst.ins.sync_info
        waits = list(si.on_wait) if si is not None and si.on_wait else []
        if len(waits) > 1:
            si.on_wait = waits[:1]
            assert self.sems is not None
            by_num = {h.num: h for h in self.sems.allocated().values()}
            for w in waits[1:]:
                d2 = nc.sync.drain()
                h = by_num.get(w.id)
                assert h is not None, f"no sem handle for wait {w.ant_name}"
                d2.wait_op(h, w.wait_value, "sem-ge", check=False)
        nc.all_engine_barrier()
        assert self.sems is not None
        popped = nc._tile_sem_poison_stack.pop()
        assert popped is self._sem_poison
        nc.clear_and_free_semaphores(list(self.sems.allocated().values()))
        nc.all_engine_barrier()

    tile.TileContext._drain_and_barrier = _drain_and_barrier
    tile.TileContext._drain_patch_installed = True


_install_antenv_shim()


def _split_multi_waits(nc, max_waits=1):
    """This walrus build rejects instructions carrying more than ~1 sync wait.
    Move excess waits onto same-engine NOPs inserted immediately before the
    instruction (sequencer waits execute in stream order, so this is
    semantics-preserving)."""
    import bass_rust
    import concourse.mybir as mybir

    n = 0
    for bb in nc.m.functions[0].blocks:
        insts = bb.instructions
        out = []
        for inst in insts:
            si = inst.sync_info
            waits = list(si.on_wait) if si is not None and si.on_wait else []
            if len(waits) > max_waits:
                keep = waits[-max_waits:]
                for w in waits[:-max_waits]:
                    nop = mybir.InstNoOp(name=f"waitnop_{n}", ins=[], outs=[])
                    n += 1
                    nop.engine = inst.engine
                    nop.sync_info = bass_rust.SyncInfo(on_wait=[w], on_update=[])
                    out.append(nop)
                si.on_wait = keep
            out.append(inst)
        if len(out) != len(insts):
            insts[:] = out
    return n


# ------------------------------------------------------------- program -----

N_CORES = 8
GROUP = 4  # cores per batch group

# Schraudolph exp-as-int16-bits: bf16_bits(exp(x)) ~= round(EXP_A*x + EXP_B).
# Calibrated for max rel err ~3.3% over x in [-4, 4]; softmax-ratio errors
# largely cancel in the attention-weighted average.
EXP_A = 128.0 / 0.6931471805599453
EXP_B = 16250.40
# exp split: even heads go to ACT; odd heads to DVE except these tb -> ACT.
ACT_ODD_TB = (5, 11)
# magic-number bf16 reciprocal seed (one Newton step follows); host negates
# Wq/bq so the seed's sign flip cancels in the final multiply.
RECIP_MAGIC = 0x7EF4  # seed bits = (den_bits - MAGIC) * -1
PACK_SCORES = True

last_results = None  # BassKernelResults of the most recent run (for test.py)


def build_program(S=2048, DM=1024, H=16, DK=64, split_waits=True):
    """Emit the SPMD Bass/Tile program. Returns nc."""
    import concourse.bass as bass
    import concourse.mybir as mybir
    import concourse.tile as tile

    _install_tile_drain_patch()

    f32 = mybir.dt.float32
    bf16 = mybir.dt.bfloat16
    i16 = mybir.dt.int16
    NPAIR = 2  # head pairs per core (4 heads)
    NH = 2 * NPAIR  # heads per core
    KT = DM // 128  # contraction chunks for projections
    TT = S // 128  # t tiles (scores row blocks / AV contraction tiles)
    SQ = min(1024, S)  # scores/exp free width (one attention block)
    NSH = S // SQ
    PB = 512  # proj s-block width
    NPB = S // PB
    HDK = H * DK  # concat dim (1024)
    KO = HDK // 128  # outproj contraction chunks
    DSL = HDK // GROUP  # out d-slice per core (256)

    nc = bass.Bass(
        trn_type="TRN2", target_bir_lowering=False, debug=False, num_devices=N_CORES
    )

    def din(name, shape, dt=bf16):
        return nc.dram_tensor(name, shape, dt, kind="ExternalInput").ap()

    # x[b].T per kind, chunk-packed [p, kc, s] = xT[kc*128+p, s] so one DMA
    # loads a [128, KT, s-block] tile
    xT = {p: din(f"x{p}T", [128, KT, S]) for p in ("q", "k", "v")}
    W = {p: din(f"w{p}", [NPAIR, 128, KT, 128]) for p in ("k", "v")}
    wq4 = din("wq4", [128, KT, NH * DK])  # all 4 heads' Wq.T, chunk-packed
    bq4 = din("bq4", [1, NH * DK])  # bq of the 4 heads (folded into q-proj)
    bk8 = din("bk8", [NPAIR, 128, 1], f32)  # bk / sqrt(dk)
    bv = din("bv", [NPAIR, 128, 1], f32)
    woT = din("woT", [128, KO, DSL])  # Wo.T columns, chunk-packed
    boT = din("boT", [128, 2], f32)  # bo d-slice as [128, 2]
    out_ap = nc.dram_tensor("out", [DSL, S], bf16, kind="ExternalOutput").ap()
    debug = bool(int(os.environ.get("BASSMHA_DEBUG", "0")))
    if debug:
        dbg_aps = {
            "d_k2T0": nc.dram_tensor("d_k2T0", [128, S], f32, kind="ExternalOutput").ap(),
            "d_v2T0": nc.dram_tensor("d_v2T0", [128, S], f32, kind="ExternalOutput").ap(),
            "d_qa0": nc.dram_tensor("d_qa0", [128, 16 * 65], f32, kind="ExternalOutput").ap(),
            "d_ho0": nc.dram_tensor("d_ho0", [128, S], f32, kind="ExternalOutput").ap(),
        }

    Exp = mybir.ActivationFunctionType.Exp
    mult = mybir.AluOpType.mult
    add = mybir.AluOpType.add
    bypass = mybir.AluOpType.bypass

    with tile.TileContext(nc) as tc:
        with contextlib.ExitStack() as ctx:
            sb = ctx.enter_context(tc.tile_pool(name="sb", bufs=2))
            big = ctx.enter_context(tc.tile_pool(name="big", bufs=8))
            ps = ctx.enter_context(tc.tile_pool(name="ps", bufs=2, space="PSUM"))
            dram = ctx.enter_context(tc.tile_pool(name="dram", bufs=1, space="DRAM"))

            # --- constants / small tiles --- (k-weights first: the first
            # matmul waits only on wk + the first x tile)
            wsb = {}
            for kind in ("k", "v"):
                wsb[kind] = [
                    sb.tile(
                        [128, KT, 128], bf16, tag="w", bufs=4, name=f"w_{kind}{p}"
                    )
                    for p in range(NPAIR)
                ]
            for p in range(NPAIR):
                nc.sync.dma_start(
                    wsb["k"][p][:, 0 : KT // 2, :], W["k"][p][:, 0 : KT // 2, :]
                )
            for p in range(NPAIR):
                nc.sync.dma_start(
                    wsb["k"][p][:, KT // 2 :, :], W["k"][p][:, KT // 2 :, :]
                )
            for p in range(NPAIR):
                nc.sync.dma_start(wsb["v"][p][:, :, :], W["v"][p])
            wq_sb = sb.tile([128, KT, NH * DK], bf16, tag="wq", bufs=1)
            nc.sync.dma_start(wq_sb[:, :, :], wq4[:, :, :])
            bk_sb = sb.tile([128, NPAIR], f32, tag="bk", bufs=1)
            bv_sb = sb.tile([128, NPAIR], f32, tag="bv", bufs=1)
            for p in range(NPAIR):
                nc.sync.dma_start(bk_sb[:, p : p + 1], bk8[p])
                nc.sync.dma_start(bv_sb[:, p : p + 1], bv[p])
            ones1r = sb.tile([1, 128], bf16, tag="o1r", bufs=1)
            nc.gpsimd.memset(ones1r[:], 1.0)
            bq4_sb = sb.tile([1, NH * DK], bf16, tag="bq4", bufs=1)
            nc.sync.dma_start(bq4_sb[:], bq4[:])

            # --- persistent big tiles ---
            k2T = [
                big.tile([128, S], bf16, tag="kv", bufs=4, name=f"k2T_{p}")
                for p in range(NPAIR)
            ]
            v2T = [
                big.tile([128, S], bf16, tag="kv", bufs=4, name=f"v2T_{p}")
                for p in range(NPAIR)
            ]
            # ones-augmented q2 per head: [t, dk|1] chunks of 65 columns per tile
            qa = [
                big.tile([128, TT * 65], bf16, tag="qa", bufs=NH, name=f"qa_{h}")
                for h in range(NH)
            ]
            for h in range(NH):
                nc.gpsimd.memset(qa[h][:], 1.0)
            headout = [
                big.tile([128, S], bf16, tag="ho", bufs=NPAIR, name=f"ho_{p}")
                for p in range(NPAIR)
            ]

            # --- phase P: projections ---
            # k/v: out[dk-pair, s] — stationary W chunk, moving xT chunk.
            # q: out[t, dk-heads] — stationary xT chunk, moving Wq of 4 heads.
            for blk in range(NPB):
                s0 = blk * PB
                for kind in ("k", "v"):
                    pv = [
                        ps.tile([128, PB], f32, tag="av", bufs=4, name=f"pv{p}")
                        for p in range(NPAIR)
                    ]
                    xt = sb.tile([128, KT, PB], bf16, tag="xt", bufs=2, name="xt")
                    nc.sync.dma_start(
                        xt[:, 0 : KT // 2, :], xT[kind][:, 0 : KT // 2, s0 : s0 + PB]
                    )
                    nc.sync.dma_start(
                        xt[:, KT // 2 :, :], xT[kind][:, KT // 2 :, s0 : s0 + PB]
                    )
                    for kc in range(KT):
                        for p in range(NPAIR):
                            nc.tensor.matmul(
                                pv[p][:],
                                wsb[kind][p][:, kc, :],
                                xt[:, kc, :],
                                start=(kc == 0),
                                stop=(kc == KT - 1),
                            )
                    for p in range(NPAIR):
                        if kind == "k":
                            nc.vector.tensor_scalar(
                                k2T[p][:, s0 : s0 + PB],
                                pv[p][:],
                                1.0 / 8.0,
                                bk_sb[:, p : p + 1],
                                mult,
                                add,
                            )
                        else:
                            nc.vector.tensor_scalar_add(
                                v2T[p][:, s0 : s0 + PB], pv[p][:], bv_sb[:, p : p + 1]
                            )
                # one accumulation region per PSUM bank: a start=True matmul
                # clears its whole bank, so each 256-wide q region gets its
                # own one-bank tile (upper 256 columns unused).
                pq = [
                    ps.tile(
                        [128, PB],
                        f32,
                        tag=("sc" if c < 2 else "av"),
                        bufs=4,
                        name=f"pq{c}",
                    )
                    for c in range(4)
                ]
                xtq = sb.tile([128, KT, PB], bf16, tag="xt", bufs=2, name="xtq")
                nc.sync.dma_start(xtq[:, :, :], xT["q"][:, :, s0 : s0 + PB])
                for kc in range(KT):
                    for c in range(4):  # t-chunks inside this s-block
                        nc.tensor.matmul(
                            pq[c][:, 0:256],
                            xtq[:, kc, c * 128 : (c + 1) * 128],
                            wq_sb[:, kc, :],
                            start=(kc == 0),
                            stop=False,
                        )
                for c in range(4):  # fold bq in: pq += ones^T (x) bq4
                    nc.tensor.matmul(
                        pq[c][:, 0:256],
                        ones1r[:],
                        bq4_sb[:],
                        start=False,
                        stop=True,
                    )
                for c in range(4):
                    tcix = blk * 4 + c
                    for h in range(NH):
                        nc.vector.tensor_copy(
                            qa[h][:, tcix * 65 : tcix * 65 + 64],
                            pq[c][:, h * 64 : h * 64 + 64],
                        )

            if debug:
                for nm, t in (("d_k2T0", k2T[0]), ("d_v2T0", v2T[0])):
                    for bkk in range(4):
                        dt_sb = sb.tile([128, 512], f32, tag="dbg", bufs=2, name="dt")
                        nc.vector.tensor_copy(dt_sb[:], t[:, bkk * 512 : (bkk + 1) * 512])
                        nc.sync.dma_start(dbg_aps[nm][:, bkk * 512 : (bkk + 1) * 512], dt_sb[:])
                for bkk in range(2):
                    dq_sb = sb.tile([128, 520], f32, tag="dbg2", bufs=2, name="dq")
                    nc.vector.tensor_copy(dq_sb[:], qa[0][:, bkk * 520 : (bkk + 1) * 520])
                    nc.sync.dma_start(dbg_aps["d_qa0"][:, bkk * 520 : (bkk + 1) * 520], dq_sb[:])

            boT_sb = sb.tile([128, 2], f32, tag="bo", bufs=1)
            nc.sync.dma_start(boT_sb[:], boT[:])
            woT_sb = sb.tile([128, KO, DSL], bf16, tag="wo", bufs=1)
            nc.sync.dma_start(woT_sb[:, :, :], woT[:, :, :])

            # --- phase A: attention over (pair, s-quarter) blocks ---
            # scores for the pair's two heads run as row-split tile_position
            # matmuls (K=64 halves of the PE array, concurrent); exp split
            # ACT/DVE; AV per head with ones-augmented q2aug; normalize is
            # DVE-only (magic reciprocal + DMA partition-broadcast), delayed
            # two blocks so the PE never waits on it.
            SQA = 512
            NSHA = S // SQA
            blocks = [(p, sh) for p in range(NPAIR) for sh in range(NSHA)]
            NB = len(blocks)
            expt = [
                [
                    [
                        big.tile(
                            [128, SQA],
                            i16,
                            tag="expt",
                            bufs=4 * TT,
                            name=f"e{par}_{hh}_{tb}",
                        )
                        for tb in range(TT)
                    ]
                    for hh in range(2)
                ]
                for par in range(2)
            ]
            av_tiles = {}
            # gathers are split into column halves: the first half of each
            # pair fires mid-attention (hidden), so only the last 1 MB
            # collective's entry latency is exposed at the tail
            SH2 = S // 2
            cc_in = [
                [dram.tile([128, SH2], bf16, name=f"cc_in_{p}_{hf}") for hf in range(2)]
                for p in range(NPAIR)
            ]
            cc_out = [
                [
                    dram.tile([GROUP * 128, SH2], bf16, name=f"cc_out_{p}_{hf}")
                    for hf in range(2)
                ]
                for p in range(NPAIR)
            ]

            def av_step(i, tk):
                p, sh = blocks[i]
                if tk == 0:
                    av_tiles[i] = [
                        ps.tile([65, SQA], f32, tag="av", bufs=4, name=f"av{hh}")
                        for hh in range(2)
                    ]
                eb = expt[i % 2]
                for hh in range(2):
                    nc.tensor.matmul(
                        av_tiles[i][hh][:],
                        qa[2 * p + hh][:, tk * 65 : tk * 65 + 65],
                        eb[hh][tk][:].bitcast(bf16),
                        start=(tk == 0),
                        stop=(tk == TT - 1),
                    )

            def normalize(i):
                p, sh = blocks[i]
                for hh in range(2):
                    prow = 64 * hh
                    av = av_tiles[i][hh]
                    # bf16 bits of the f32 denominator = its high bytes; read
                    # them straight out of PSUM with a stride-2 int16 view
                    r0 = sb.tile([1, SQA], i16, tag="r0", bufs=4, name="r0")
                    nc.vector.tensor_scalar(
                        r0[:],
                        av[64:65, :].bitcast(i16)[:, 1::2],
                        RECIP_MAGIC,
                        -1,
                        mybir.AluOpType.subtract,
                        mult,
                    )
                    t1 = sb.tile([1, SQA], f32, tag="t1", bufs=4, name="t1")
                    nc.vector.tensor_mul(t1[:], av[64:65, :], r0[:].bitcast(bf16))
                    rec = sb.tile([1, SQA], f32, tag="rec", bufs=4, name="rec")
                    nc.vector.scalar_tensor_tensor(
                        rec[:],
                        t1[:],
                        2.0,
                        r0[:].bitcast(bf16),
                        mybir.AluOpType.subtract,
                        mult,
                    )
                    rdr = dram.tile([1, SQA], f32, name=f"rdr_{i}_{hh}")
                    nc.sync.dma_start(rdr[:], rec[:])
                    bcb = sb.tile([64, SQA], f32, tag="bcb", bufs=4, name="bcb")
                    s2b, _ = bass.broadcast_tensor_aps(rdr[:], bcb[:])
                    nc.sync.dma_start(bcb[:], s2b)
                    dst = headout[p][prow : prow + 64, sh * SQA : (sh + 1) * SQA]
                    nc.vector.tensor_mul(dst, av[0:64, :], bcb[:])
                    nc.sync.dma_start(
                        cc_in[p][sh // 2][
                            prow : prow + 64, (sh % 2) * SQA : (sh % 2 + 1) * SQA
                        ],
                        dst,
                    )
                if sh % 2 == 1:
                    hf = sh // 2
                    nc.gpsimd.collective_compute(
                        "AllGather",
                        mybir.AluOpType.bypass,
                        replica_groups=[[0, 1, 2, 3], [4, 5, 6, 7]],
                        ins=[cc_in[p][hf].opt()],
                        outs=[cc_out[p][hf].opt()],
                    )
                del av_tiles[i]

            for j in range(NB):
                p, sh = blocks[j]
                eb = expt[j % 2]
                for tb in range(TT):
                    for hh in range(2):
                        sc = ps.tile(
                            [128, SQA], f32, tag="sc", bufs=4, name=f"sc{hh}"
                        )
                        nc.tensor.matmul(
                            sc[:],
                            k2T[p][64 * hh : 64 * hh + 64, tb * 128 : (tb + 1) * 128],
                            v2T[p][64 * hh : 64 * hh + 64, sh * SQA : (sh + 1) * SQA],
                            start=True,
                            stop=True,
                            tile_position=((64 * hh, 0) if PACK_SCORES else None),
                        )
                        if hh == 0 or tb in ACT_ODD_TB:
                            nc.scalar.activation(
                                eb[hh][tb][:].bitcast(bf16), sc[:], Exp
                            )
                        else:
                            nc.vector.tensor_scalar(
                                eb[hh][tb][:], sc[:], EXP_A, EXP_B, mult, add
                            )
                    if j > 0:
                        av_step(j - 1, tb)
                if j > 0:
                    normalize(j - 1)
            for tk in range(TT):
                av_step(NB - 1, tk)
            normalize(NB - 1)

            if debug:
                for bkk in range(4):
                    dh_sb = sb.tile([128, 512], f32, tag="dbg", bufs=2, name="dh")
                    nc.vector.tensor_copy(dh_sb[:], headout[0][:, bkk * 512 : (bkk + 1) * 512])
                    nc.sync.dma_start(dbg_aps["d_ho0"][:, bkk * 512 : (bkk + 1) * 512], dh_sb[:])

            # --- phase O: output projection, transposed layout outT[d, s] ---
            # k-major: all pair-0 contraction chunks (ready after the first
            # AllGather) stream through every output tile before any pair-1
            # chunk, so the PE crunches during the second AllGather.  All 8
            # [128, 512] accumulation tiles live at once (full PSUM).
            korder = [k for k in range(KO) if k % 2 == 0] + [
                k for k in range(KO) if k % 2 == 1
            ]
            OSW = min(512, S)
            NSB = S // OSW
            pos = {}
            for sblk in range(NSB):
                for dblk in range(2):
                    pos[(sblk, dblk)] = ps.tile(
                        [128, OSW],
                        f32,
                        tag=("sc" if sblk < 2 else "av"),
                        bufs=4,
                        name=f"po{sblk}{dblk}",
                    )
            def drain_pos(sblk, dblk):
                ob = sb.tile([128, OSW], bf16, tag="ob", bufs=3, name="ob")
                nc.vector.tensor_scalar_add(
                    ob[:], pos[(sblk, dblk)][:], boT_sb[:, dblk : dblk + 1]
                )
                nc.sync.dma_start(
                    out_ap[
                        128 * dblk : 128 * (dblk + 1), sblk * OSW : (sblk + 1) * OSW
                    ],
                    ob[:],
                )

            for ki, k in enumerate(korder):
                # separate half-tiles: s-blocks 0-1 depend only on the early
                # half-gather, so their matmuls run during the last collective
                for hf in range(2):
                    chh = sb.tile([128, SH2], bf16, tag="ch", bufs=4, name="chh")
                    nc.sync.dma_start(
                        chh[:],
                        cc_out[k % 2][hf][128 * (k // 2) : 128 * (k // 2) + 128, :],
                    )
                    for sblk in (0, 1) if hf == 0 else (2, 3):
                        for dblk in range(2):
                            nc.tensor.matmul(
                                pos[(sblk, dblk)][:],
                                woT_sb[:, k, 128 * dblk : 128 * (dblk + 1)],
                                chh[:, (sblk % 2) * OSW : (sblk % 2 + 1) * OSW],
                                start=(ki == 0),
                                stop=(ki == KO - 1),
                            )
                            if ki == KO - 1:
                                drain_pos(sblk, dblk)

    if split_waits:
        _split_multi_waits(nc)
    return nc


def make_in_maps(v, k, q, Wq, bqv, Wk, bkv, Wv, bvv, Wo, bov, S, DM, H, DK):
    """Per-core input dicts from full inputs (host-side prep is slicing /
    transpose / dtype conversion).  Wq/bq are NEGATED: the magic-reciprocal
    in the kernel produces -1/den, and the two sign flips cancel."""
    import ml_dtypes

    bfdt = ml_dtypes.bfloat16
    HDK = H * DK
    DSL = HDK // GROUP
    KT = DM // 128

    def chunkpack(a):  # [DM, N] -> [128, KT, N]
        return np.ascontiguousarray(
            a.reshape(KT, 128, a.shape[1]).transpose(1, 0, 2)
        ).astype(bfdt)

    xT = {}
    for b in range(2):
        xT[("q", b)] = chunkpack(q[b].T)
        xT[("k", b)] = chunkpack(k[b].T)
        xT[("v", b)] = chunkpack(v[b].T)
    WoT = np.ascontiguousarray(Wo.T)  # [HDK, HDK_out]
    in_maps = []
    for c in range(N_CORES):
        b = c // GROUP
        h0 = 4 * (c % GROUP)
        m = {
            "xqT": xT[("q", b)],
            "xkT": xT[("k", b)],
            "xvT": xT[("v", b)],
        }
        for kind, Wt, bt in (("k", Wk, bkv), ("v", Wv, bvv)):
            wp = np.empty((2, DM, 128), np.float32)
            bp = np.empty((2, 128, 1), np.float32)
            for p in range(2):
                ha, hb = h0 + 2 * p, h0 + 2 * p + 1
                wp[p, :, :64] = Wt[ha].T
                wp[p, :, 64:] = Wt[hb].T
                bp[p, :64, 0] = bt[ha]
                bp[p, 64:, 0] = bt[hb]
            m[f"w{kind}"] = np.ascontiguousarray(
                wp.reshape(2, KT, 128, 128).transpose(0, 2, 1, 3)
            ).astype(bfdt)
            if kind == "k":
                m["bk8"] = bp / 8.0
            else:
                m["bv"] = bp
        m["wq4"] = chunkpack(
            -np.concatenate([Wq[h0 + h].T for h in range(4)], axis=1)
        )
        m["bq4"] = (
            -np.concatenate([bqv[h0 + h] for h in range(4)])
            .reshape(1, -1)
            .astype(bfdt)
        )
        d0 = DSL * (c % GROUP)
        m["woT"] = chunkpack(WoT[:, d0 : d0 + DSL])
        m["boT"] = np.ascontiguousarray(bov[d0 : d0 + DSL].reshape(2, 128).T)
        in_maps.append(m)
    return in_maps


def kernel(v, k, q, Wq, bq, Wk, bk, Wv, bv, Wo, bo, _trace=False):
    """Full inputs in, full output out. Runs the SPMD Bass kernel on 8 cores."""
    global last_results
    from concourse.bass_utils import run_bass_kernel_spmd

    v, k, q = (np.asarray(a, np.float32) for a in (v, k, q))
    B, S, DM = q.shape
    H, DK = Wq.shape[0], Wq.shape[1]
    HDK = H * DK
    DSL = HDK // GROUP

    nc = build_program(S=S, DM=DM, H=H, DK=DK)
    in_maps = make_in_maps(
        v,
        k,
        q,
        *(np.asarray(a, np.float32) for a in (Wq, bq, Wk, bk, Wv, bv, Wo, bo)),
        S=S,
        DM=DM,
        H=H,
        DK=DK,
    )
    res = run_bass_kernel_spmd(nc, in_maps, list(range(N_CORES)), trace=_trace)
    last_results = res
    out = np.empty((B, S, HDK), np.float32)
    for c in range(N_CORES):
        b = c // GROUP
        d0 = DSL * (c % GROUP)
        out[b, :, d0 : d0 + DSL] = res.results[c]["out"].astype(np.float32).T
    return out



# revision 3
# speedup vs baseline: 1.0514x; 1.0514x over previous
"""Trainium2 Bass kernel for nn_MultiHeadAttention_56118042690041.

8-core sharding: batch x heads tensor-parallel.
  core c (0..7): batch b = c//4, heads 4*(c%4) .. 4*(c%4)+4 (as 2 packed pairs).
Per core (all matmul operands bf16, PSUM accumulation f32):
  - QKV projections. k/v: pair-packed stationary W, moving xT -> k2T/v2T in
    [dk, s] layout. q: flipped orientation (stationary xT chunk, moving W of
    all 4 heads) -> q2 directly in [t, dk] layout, assembled into
    ones-augmented per-head q2aug tiles (no PE transposes needed).
    xT is block-major packed ([128, blk, kc, s']) so each tile DMA is a
    4KB-contiguous-per-partition transfer; first-needed DMAs issue first.
  - Attention per head (note reference's faithful "bug": scores = v2 @ k2^T,
    softmax, weighted sum of q2): software-pipelined blocks of (pair, s-qtr)
    in pair-alternating order: scores block j interleaved tb-wise with the AV
    matmuls of block j-1 so the PE never waits on exp. exp is split between
    ACT (exact, bf16 out) and DVE (Schraudolph bit-trick: round(A*x+B) ->
    int16, bitcast bf16; softmax-ratio errors cancel in the weighted
    average). AV uses a ones-column augmented q2aug so the softmax
    denominator falls out of the same matmul. Normalization is a 3-stage
    pipeline: reciprocal_approx_fast on the denominator row (block j-1) +
    DMA partition-broadcast, then the [64, s] multiply + per-(pair, s-qtr)
    AllGather one block later (block j-2), so neither DVE nor the PE ever
    waits on the broadcast round trip and the 8 small collectives spread
    evenly through the attention phase.
  - Output projection per s-quarter: each core computes a disjoint 256-wide
    d-slice of out = headout @ Wo^T + bo (column-sharded Wo); quarter q's
    matmuls start as soon as its two gathers have landed, so only the last
    quarter's gather latency is exposed.
Host: slices/packs weights per core (bf16), transposes x (bf16), concatenates
disjoint outputs.
"""

import contextlib
import ctypes
import os
import sys
import types

import numpy as np

if "/opt/trn_rl_repo" not in sys.path:
    sys.path.insert(0, "/opt/trn_rl_repo")

# ---------------------------------------------------------------- shims ----


def _install_antenv_shim():
    """Provide antenv.axon_hooks (NTFF profile hook) if the image lacks it."""
    try:
        import antenv.axon_hooks  # noqa: F401

        return
    except ImportError:
        pass

    def _hook_factory():
        so_path = "/opt/axon/libaxon_pjrt.so"
        try:
            lib = ctypes.CDLL(so_path)
        except OSError:
            return None
        if not hasattr(lib, "axon_start_nrt_profile"):
            return None
        lib.axon_start_nrt_profile.argtypes = [
            ctypes.POINTER(ctypes.c_int64),
            ctypes.c_size_t,
        ]
        lib.axon_start_nrt_profile.restype = ctypes.c_int64
        lib.axon_stop_nrt_profile.argtypes = [ctypes.c_char_p]
        lib.axon_stop_nrt_profile.restype = ctypes.c_int64

        @contextlib.contextmanager
        def _hook(output_dir, device_ids):
            import jax

            jax.devices()
            if device_ids:
                ids = (ctypes.c_int64 * len(device_ids))(*device_ids)
                rc = lib.axon_start_nrt_profile(ids, len(device_ids))
            else:
                rc = lib.axon_start_nrt_profile(None, 0)
            if rc != 0:
                raise RuntimeError(f"axon_start_nrt_profile rc={rc}")
            try:
                yield
            finally:
                n = lib.axon_stop_nrt_profile(str(output_dir).encode())
                print(f"ntff profile: {n} file(s) -> {output_dir}", file=sys.stderr)

        return _hook

    hook = _hook_factory()
    mod = types.ModuleType("antenv.axon_hooks")
    mod.get_axon_ntff_profile_hook = lambda: hook
    mod.set_axon_ntff_profile_hook = lambda h: None
    sys.modules["antenv.axon_hooks"] = mod


def _install_tile_drain_patch():
    """This walrus build rejects >1 sync wait on the Tile tail Drain; split the
    waits across chained single-wait drains."""
    import concourse.tile as tile

    if getattr(tile.TileContext, "_drain_patch_installed", False):
        return

    def _drain_and_barrier(self, tick_clock, wait_clock):
        nc = self.nc
        drain_inst = nc.sync.drain()
        wait_clock.add_sem_waits(
            drain_inst.ins, tile.ScopedClock({None: tick_clock.global_clock})
        )
        si = drain_inst.ins.sync_info
        waits = list(si.on_wait) if si is not None and si.on_wait else []
        if len(waits) > 1:
            si.on_wait = waits[:1]
            assert self.sems is not None
            by_num = {h.num: h for h in self.sems.allocated().values()}
            for w in waits[1:]:
                d2 = nc.sync.drain()
                h = by_num.get(w.id)
                assert h is not None, f"no sem handle for wait {w.ant_name}"
                d2.wait_op(h, w.wait_value, "sem-ge", check=False)
        nc.all_engine_barrier()
        assert self.sems is not None
        popped = nc._tile_sem_poison_stack.pop()
        assert popped is self._sem_poison
        nc.clear_and_free_semaphores(list(self.sems.allocated().values()))
        nc.all_engine_barrier()

    tile.TileContext._drain_and_barrier = _drain_and_barrier
    tile.TileContext._drain_patch_installed = True


_install_antenv_shim()


def _split_multi_waits(nc, max_waits=1):
    """This walrus build rejects instructions carrying more than ~1 sync wait.
    Move excess waits onto same-engine NOPs inserted immediately before the
    instruction (sequencer waits execute in stream order, so this is
    semantics-preserving)."""
    import bass_rust
    import concourse.mybir as mybir

    n = 0
    for bb in nc.m.functions[0].blocks:
        insts = bb.instructions
        out = []
        for inst in insts:
            si = inst.sync_info
            waits = list(si.on_wait) if si is not None and si.on_wait else []
            if len(waits) > max_waits:
                keep = waits[-max_waits:]
                for w in waits[:-max_waits]:
                    nop = mybir.InstNoOp(name=f"waitnop_{n}", ins=[], outs=[])
                    n += 1
                    nop.engine = inst.engine
                    nop.sync_info = bass_rust.SyncInfo(on_wait=[w], on_update=[])
                    out.append(nop)
                si.on_wait = keep
            out.append(inst)
        if len(out) != len(insts):
            insts[:] = out
    return n


# ------------------------------------------------------------- program -----

N_CORES = 8
GROUP = 4  # cores per batch group

# Schraudolph exp-as-int16-bits: bf16_bits(exp(x)) ~= round(EXP_A*x + EXP_B).
# Calibrated for max rel err ~3.3% over x in [-4, 4]; softmax-ratio errors
# largely cancel in the attention-weighted average.
EXP_A = 128.0 / 0.6931471805599453
EXP_B = 16250.40
# exp split: even heads go to ACT; odd heads to DVE except these tb -> ACT.
ACT_ODD_TB = (5, 11)

last_results = None  # BassKernelResults of the most recent run (for test.py)


def build_program(S=2048, DM=1024, H=16, DK=64, split_waits=True):
    """Emit the SPMD Bass/Tile program. Returns nc."""
    import concourse.bass as bass
    import concourse.mybir as mybir
    import concourse.tile as tile

    _install_tile_drain_patch()

    f32 = mybir.dt.float32
    bf16 = mybir.dt.bfloat16
    i16 = mybir.dt.int16
    NPAIR = 2  # head pairs per core (4 heads)
    NH = 2 * NPAIR  # heads per core
    KT = DM // 128  # contraction chunks for projections
    TT = S // 128  # t tiles (scores row blocks / AV contraction tiles)
    PB = 512  # proj s-block width
    NPB = S // PB
    HDK = H * DK  # concat dim (1024)
    KO = HDK // 128  # outproj contraction chunks
    DSL = HDK // GROUP  # out d-slice per core (256)

    nc = bass.Bass(
        trn_type="TRN2", target_bir_lowering=False, debug=False, num_devices=N_CORES
    )

    def din(name, shape, dt=bf16):
        return nc.dram_tensor(name, shape, dt, kind="ExternalInput").ap()

    # x[b].T per kind, block-major chunk-packed [p, blk, kc, s'] =
    # xT[kc*128+p, blk*PB+s'] so one tile DMA is 4KB-contiguous per partition
    xT = {p: din(f"x{p}T", [128, NPB, KT, PB]) for p in ("q", "k", "v")}
    W = {p: din(f"w{p}", [NPAIR, 128, KT, 128]) for p in ("k", "v")}
    wq4 = din("wq4", [128, KT, NH * DK])  # all 4 heads' Wq.T, chunk-packed
    bq4 = din("bq4", [1, NH * DK])  # bq of the 4 heads (folded into q-proj)
    bk8 = din("bk8", [NPAIR, 128, 1], f32)  # bk / sqrt(dk)
    bv = din("bv", [NPAIR, 128, 1], f32)
    woT = din("woT", [128, KO, DSL])  # Wo.T columns, chunk-packed
    boT = din("boT", [128, 2], f32)  # bo d-slice as [128, 2]
    out_ap = nc.dram_tensor("out", [DSL, S], bf16, kind="ExternalOutput").ap()

    Exp = mybir.ActivationFunctionType.Exp
    mult = mybir.AluOpType.mult
    add = mybir.AluOpType.add

    with tile.TileContext(nc) as tc:
        with contextlib.ExitStack() as ctx:
            sb = ctx.enter_context(tc.tile_pool(name="sb", bufs=2))
            big = ctx.enter_context(tc.tile_pool(name="big", bufs=8))
            ps = ctx.enter_context(tc.tile_pool(name="ps", bufs=2, space="PSUM"))
            dram = ctx.enter_context(tc.tile_pool(name="dram", bufs=1, space="DRAM"))

            # --- startup DMAs in consumption order: the first k-proj matmul
            # waits only on wk + the first xk tile ---
            wsb = {}
            for kind in ("k", "v"):
                wsb[kind] = [
                    sb.tile(
                        [128, KT, 128], bf16, tag="w", bufs=4, name=f"w_{kind}{p}"
                    )
                    for p in range(NPAIR)
                ]
            for p in range(NPAIR):
                nc.sync.dma_start(wsb["k"][p][:, :, :], W["k"][p])

            def dma_xt(kind, blk):
                t = sb.tile([128, KT, PB], bf16, tag="xt", bufs=3, name=f"xt_{kind}")
                nc.sync.dma_start(
                    t[:, 0 : KT // 2, :], xT[kind][:, blk, 0 : KT // 2, :]
                )
                nc.sync.dma_start(t[:, KT // 2 :, :], xT[kind][:, blk, KT // 2 :, :])
                return t

            xt0 = {"k": dma_xt("k", 0)}
            for p in range(NPAIR):
                nc.sync.dma_start(wsb["v"][p][:, :, :], W["v"][p])
            xt0["v"] = dma_xt("v", 0)
            bk_sb = sb.tile([128, NPAIR], f32, tag="bk", bufs=1)
            bv_sb = sb.tile([128, NPAIR], f32, tag="bv", bufs=1)
            for p in range(NPAIR):
                nc.sync.dma_start(bk_sb[:, p : p + 1], bk8[p])
                nc.sync.dma_start(bv_sb[:, p : p + 1], bv[p])
            wq_sb = sb.tile([128, KT, NH * DK], bf16, tag="wq", bufs=1)
            nc.sync.dma_start(wq_sb[:, :, :], wq4[:, :, :])
            ones1r = sb.tile([1, 128], bf16, tag="o1r", bufs=1)
            nc.gpsimd.memset(ones1r[:], 1.0)
            bq4_sb = sb.tile([1, NH * DK], bf16, tag="bq4", bufs=1)
            nc.sync.dma_start(bq4_sb[:], bq4[:])

            # --- persistent big tiles ---
            k2T = [
                big.tile([128, S], bf16, tag="kv", bufs=4, name=f"k2T_{p}")
                for p in range(NPAIR)
            ]
            v2T = [
                big.tile([128, S], bf16, tag="kv", bufs=4, name=f"v2T_{p}")
                for p in range(NPAIR)
            ]
            # ones-augmented q2 per head: [t, dk|1] chunks of 65 columns per tile
            qa = [
                big.tile([128, TT * 65], bf16, tag="qa", bufs=NH, name=f"qa_{h}")
                for h in range(NH)
            ]
            for h in range(NH):
                nc.gpsimd.memset(qa[h][:], 1.0)
            headout = [
                big.tile([128, S], bf16, tag="ho", bufs=NPAIR, name=f"ho_{p}")
                for p in range(NPAIR)
            ]

            # --- phase P: projections ---
            # k/v: out[dk-pair, s] — stationary W chunk, moving xT chunk.
            # q: out[t, dk-heads] — stationary xT chunk, moving Wq of 4 heads.
            for blk in range(NPB):
                s0 = blk * PB
                for kind in ("k", "v"):
                    pv = [
                        ps.tile([128, PB], f32, tag="av", bufs=4, name=f"pv{p}")
                        for p in range(NPAIR)
                    ]
                    xt = xt0[kind] if blk == 0 else dma_xt(kind, blk)
                    for kc in range(KT):
                        for p in range(NPAIR):
                            nc.tensor.matmul(
                                pv[p][:],
                                wsb[kind][p][:, kc, :],
                                xt[:, kc, :],
                                start=(kc == 0),
                                stop=(kc == KT - 1),
                            )
                    for p in range(NPAIR):
                        if kind == "k":
                            nc.vector.tensor_scalar(
                                k2T[p][:, s0 : s0 + PB],
                                pv[p][:],
                                1.0 / 8.0,
                                bk_sb[:, p : p + 1],
                                mult,
                                add,
                            )
                        else:
                            nc.vector.tensor_scalar_add(
                                v2T[p][:, s0 : s0 + PB], pv[p][:], bv_sb[:, p : p + 1]
                            )
                # one accumulation region per PSUM bank: a start=True matmul
                # clears its whole bank, so each 256-wide q region gets its
                # own one-bank tile (upper 256 columns unused).
                pq = [
                    ps.tile(
                        [128, PB],
                        f32,
                        tag=("sc" if c < 2 else "av"),
                        bufs=4,
                        name=f"pq{c}",
                    )
                    for c in range(4)
                ]
                xtq = dma_xt("q", blk)
                for kc in range(KT):
                    for c in range(4):  # t-chunks inside this s-block
                        nc.tensor.matmul(
                            pq[c][:, 0:256],
                            xtq[:, kc, c * 128 : (c + 1) * 128],
                            wq_sb[:, kc, :],
                            start=(kc == 0),
                            stop=False,
                        )
                for c in range(4):  # fold bq in: pq += ones^T (x) bq4
                    nc.tensor.matmul(
                        pq[c][:, 0:256],
                        ones1r[:],
                        bq4_sb[:],
                        start=False,
                        stop=True,
                    )
                for c in range(4):
                    tcix = blk * 4 + c
                    for h in range(NH):
                        nc.vector.tensor_copy(
                            qa[h][:, tcix * 65 : tcix * 65 + 64],
                            pq[c][:, h * 64 : h * 64 + 64],
                        )

            boT_sb = sb.tile([128, 2], f32, tag="bo", bufs=1)
            nc.sync.dma_start(boT_sb[:], boT[:])
            woT_sb = sb.tile([128, KO, DSL], bf16, tag="wo", bufs=1)
            nc.sync.dma_start(woT_sb[:, :, :], woT[:, :, :])

            # --- phase A: attention over (pair, s-quarter) blocks, pair-
            # alternating so the per-(pair, s-qtr) gathers spread evenly ---
            # scores for the pair's two heads run as row-split tile_position
            # matmuls (K=64 halves of the PE array, concurrent); exp split
            # ACT/DVE; AV per head with ones-augmented q2aug; normalize is a
            # 3-stage pipeline (recip at +1 block, multiply+gather at +2) so
            # no engine waits on the broadcast DMA round trip.
            SQA = 512
            NSHA = S // SQA
            blocks = [(p, sh) for sh in range(NSHA) for p in range(NPAIR)]
            NB = len(blocks)
            expt = [
                [
                    [
                        big.tile(
                            [128, SQA],
                            i16,
                            tag="expt",
                            bufs=4 * TT,
                            name=f"e{par}_{hh}_{tb}",
                        )
                        for tb in range(TT)
                    ]
                    for hh in range(2)
                ]
                for par in range(2)
            ]
            av_tiles = {}
            rec_bufs = {}
            cc_in = [
                [dram.tile([128, SQA], bf16, name=f"cc_in_{p}_{sh}") for sh in range(NSHA)]
                for p in range(NPAIR)
            ]
            cc_out = [
                [
                    dram.tile([GROUP * 128, SQA], bf16, name=f"cc_out_{p}_{sh}")
                    for sh in range(NSHA)
                ]
                for p in range(NPAIR)
            ]

            def av_step(i, tk):
                p, sh = blocks[i]
                if tk == 0:
                    av_tiles[i] = [
                        ps.tile([65, SQA], f32, tag="av", bufs=4, name=f"av{hh}")
                        for hh in range(2)
                    ]
                eb = expt[i % 2]
                for hh in range(2):
                    nc.tensor.matmul(
                        av_tiles[i][hh][:],
                        qa[2 * p + hh][:, tk * 65 : tk * 65 + 65],
                        eb[hh][tk][:].bitcast(bf16),
                        start=(tk == 0),
                        stop=(tk == TT - 1),
                    )

            def emit_recip(i):
                # stage 1: reciprocal of the denominator row + DMA broadcast
                # to 64 partitions; consumed by emit_mul_cc one block later.
                rec_bufs[i] = []
                for hh in range(2):
                    av = av_tiles[i][hh]
                    rec = sb.tile([1, SQA], f32, tag="rec", bufs=4, name="rec")
                    nc.vector.reciprocal(rec[:], av[64:65, :])
                    rdr = dram.tile([1, SQA], f32, name=f"rdr_{i}_{hh}")
                    nc.sync.dma_start(rdr[:], rec[:])
                    bcb = sb.tile([64, SQA], f32, tag="bcb", bufs=4, name="bcb")
                    s2b, _ = bass.broadcast_tensor_aps(rdr[:], bcb[:])
                    nc.sync.dma_start(bcb[:], s2b)
                    rec_bufs[i].append(bcb)

            def emit_mul_cc(i):
                # stage 2: normalize multiply, stage headout, fire the gather
                p, sh = blocks[i]
                for hh in range(2):
                    prow = 64 * hh
                    av = av_tiles[i][hh]
                    dst = headout[p][prow : prow + 64, sh * SQA : (sh + 1) * SQA]
                    nc.vector.tensor_mul(dst, av[0:64, :], rec_bufs[i][hh][:])
                    nc.sync.dma_start(cc_in[p][sh][prow : prow + 64, :], dst)
                nc.gpsimd.collective_compute(
                    "AllGather",
                    mybir.AluOpType.bypass,
                    replica_groups=[[0, 1, 2, 3], [4, 5, 6, 7]],
                    ins=[cc_in[p][sh].opt()],
                    outs=[cc_out[p][sh].opt()],
                )
                del av_tiles[i]
                del rec_bufs[i]

            for j in range(NB):
                p, sh = blocks[j]
                eb = expt[j % 2]
                for tb in range(TT):
                    for hh in range(2):
                        sc = ps.tile(
                            [128, SQA], f32, tag="sc", bufs=4, name=f"sc{hh}"
                        )
                        nc.tensor.matmul(
                            sc[:],
                            k2T[p][64 * hh : 64 * hh + 64, tb * 128 : (tb + 1) * 128],
                            v2T[p][64 * hh : 64 * hh + 64, sh * SQA : (sh + 1) * SQA],
                            start=True,
                            stop=True,
                            tile_position=(64 * hh, 0),
                        )
                        if hh == 0 or tb in ACT_ODD_TB:
                            nc.scalar.activation(
                                eb[hh][tb][:].bitcast(bf16), sc[:], Exp
                            )
                        else:
                            nc.vector.tensor_scalar(
                                eb[hh][tb][:], sc[:], EXP_A, EXP_B, mult, add
                            )
                    if j > 0:
                        av_step(j - 1, tb)
                if j > 0:
                    emit_recip(j - 1)
                if j > 1:
                    emit_mul_cc(j - 2)
            for tk in range(TT):
                av_step(NB - 1, tk)
            emit_recip(NB - 1)
            emit_mul_cc(NB - 2)
            emit_mul_cc(NB - 1)

            # --- phase O: output projection per s-quarter, transposed layout
            # outT[d, s]. Quarter q depends only on gathers (0, q) and (1, q);
            # quarters 0-2 are ready well before the attention tail, so only
            # the last quarter's gather latency is exposed. ---
            OSW = SQA
            for sblk in range(NSHA):
                pos = [
                    ps.tile([128, OSW], f32, tag="sc", bufs=4, name=f"po{dblk}")
                    for dblk in range(2)
                ]
                for ci, (p, g) in enumerate(
                    [(p, g) for p in range(NPAIR) for g in range(GROUP)]
                ):
                    chh = sb.tile([128, OSW], bf16, tag="ch", bufs=4, name="chh")
                    nc.sync.dma_start(
                        chh[:], cc_out[p][sblk][128 * g : 128 * g + 128, :]
                    )
                    for dblk in range(2):
                        nc.tensor.matmul(
                            pos[dblk][:],
                            woT_sb[:, 2 * g + p, 128 * dblk : 128 * (dblk + 1)],
                            chh[:],
                            start=(ci == 0),
                            stop=(ci == KO - 1),
                        )
                for dblk in range(2):
                    ob = sb.tile([128, OSW], bf16, tag="ob", bufs=3, name="ob")
                    nc.vector.tensor_scalar_add(
                        ob[:], pos[dblk][:], boT_sb[:, dblk : dblk + 1]
                    )
                    nc.sync.dma_start(
                        out_ap[
                            128 * dblk : 128 * (dblk + 1),
                            sblk * OSW : (sblk + 1) * OSW,
                        ],
                        ob[:],
                    )

    if split_waits:
        _split_multi_waits(nc)
    return nc


def make_in_maps(v, k, q, Wq, bqv, Wk, bkv, Wv, bvv, Wo, bov, S, DM, H, DK):
    """Per-core input dicts from full inputs (host-side prep is slicing /
    transpose / dtype conversion)."""
    import ml_dtypes

    bfdt = ml_dtypes.bfloat16
    HDK = H * DK
    DSL = HDK // GROUP
    KT = DM // 128
    PB = 512
    NPB = S // PB

    def chunkpack(a):  # [DM, N] -> [128, KT, N]
        return np.ascontiguousarray(
            a.reshape(KT, 128, a.shape[1]).transpose(1, 0, 2)
        ).astype(bfdt)

    def blockpack(a):  # [DM, S] -> [128, NPB, KT, PB] (block-major)
        return np.ascontiguousarray(
            a.reshape(KT, 128, NPB, PB).transpose(1, 2, 0, 3)
        ).astype(bfdt)

    xT = {}
    for b in range(2):
        xT[("q", b)] = blockpack(q[b].T)
        xT[("k", b)] = blockpack(k[b].T)
        xT[("v", b)] = blockpack(v[b].T)
    WoT = np.ascontiguousarray(Wo.T)  # [HDK, HDK_out]
    in_maps = []
    for c in range(N_CORES):
        b = c // GROUP
        h0 = 4 * (c % GROUP)
        m = {
            "xqT": xT[("q", b)],
            "xkT": xT[("k", b)],
            "xvT": xT[("v", b)],
        }
        for kind, Wt, bt in (("k", Wk, bkv), ("v", Wv, bvv)):
            wp = np.empty((2, DM, 128), np.float32)
            bp = np.empty((2, 128, 1), np.float32)
            for p in range(2):
                ha, hb = h0 + 2 * p, h0 + 2 * p + 1
                wp[p, :, :64] = Wt[ha].T
                wp[p, :, 64:] = Wt[hb].T
                bp[p, :64, 0] = bt[ha]
                bp[p, 64:, 0] = bt[hb]
            m[f"w{kind}"] = np.ascontiguousarray(
                wp.reshape(2, KT, 128, 128).transpose(0, 2, 1, 3)
            ).astype(bfdt)
            if kind == "k":
                m["bk8"] = bp / 8.0
            else:
                m["bv"] = bp
        m["wq4"] = chunkpack(
            np.concatenate([Wq[h0 + h].T for h in range(4)], axis=1)
        )
        m["bq4"] = (
            np.concatenate([bqv[h0 + h] for h in range(4)])
            .reshape(1, -1)
            .astype(bfdt)
        )
        d0 = DSL * (c % GROUP)
        m["woT"] = chunkpack(WoT[:, d0 : d0 + DSL])
        m["boT"] = np.ascontiguousarray(bov[d0 : d0 + DSL].reshape(2, 128).T)
        in_maps.append(m)
    return in_maps


def kernel(v, k, q, Wq, bq, Wk, bk, Wv, bv, Wo, bo, _trace=False):
    """Full inputs in, full output out. Runs the SPMD Bass kernel on 8 cores."""
    global last_results
    from concourse.bass_utils import run_bass_kernel_spmd

    v, k, q = (np.asarray(a, np.float32) for a in (v, k, q))
    B, S, DM = q.shape
    H, DK = Wq.shape[0], Wq.shape[1]
    HDK = H * DK
    DSL = HDK // GROUP

    nc = build_program(S=S, DM=DM, H=H, DK=DK)
    in_maps = make_in_maps(
        v,
        k,
        q,
        *(np.asarray(a, np.float32) for a in (Wq, bq, Wk, bk, Wv, bv, Wo, bo)),
        S=S,
        DM=DM,
        H=H,
        DK=DK,
    )
    res = run_bass_kernel_spmd(nc, in_maps, list(range(N_CORES)), trace=_trace)
    last_results = res
    out = np.empty((B, S, HDK), np.float32)
    for c in range(N_CORES):
        b = c // GROUP
        d0 = DSL * (c % GROUP)
        out[b, :, d0 : d0 + DSL] = res.results[c]["out"].astype(np.float32).T
    return out


# revision 40
# speedup vs baseline: 1.1677x; 1.1106x over previous
"""Trainium2 Bass kernel for nn_MultiHeadAttention_56118042690041.

8-core sharding: batch x heads tensor-parallel.
  core c (0..7): batch b = c//4, heads 4*(c%4) .. 4*(c%4)+4 (as 2 packed pairs).
Per core (all matmul operands bf16, PSUM accumulation f32):
  - QKV projections. k/v: pair-packed stationary W, moving xT -> k2T/v2T in
    [dk, s] layout. q: flipped orientation (stationary xT chunk, moving W of
    all 4 heads) -> q2 directly in [t, dk] layout, assembled into
    ones-augmented per-head q2aug tiles (no PE transposes needed).
    xT is block-major packed ([128, blk, kc, s']) so each tile DMA is a
    4KB-contiguous-per-partition transfer; first-needed DMAs issue first.
  - Attention per head (note reference's faithful "bug": scores = v2 @ k2^T,
    softmax, weighted sum of q2): software-pipelined blocks of (pair, s-qtr)
    in pair-alternating order: scores block j interleaved tb-wise with the AV
    matmuls of block j-1 so the PE never waits on exp. exp is split between
    ACT (exact, bf16 out) and DVE (Schraudolph bit-trick: round(A*x+B) ->
    int16, bitcast bf16; softmax-ratio errors cancel in the weighted
    average). AV uses a ones-column augmented q2aug so the softmax
    denominator falls out of the same matmul. Normalization is a 3-stage
    pipeline: reciprocal_approx_fast on the denominator row (block j-1) +
    DMA partition-broadcast, then the [64, s] multiply + per-(pair, s-qtr)
    AllGather one block later (block j-2), so neither DVE nor the PE ever
    waits on the broadcast round trip and the 8 small collectives spread
    evenly through the attention phase.
  - Output projection per s-quarter: each core computes a disjoint 256-wide
    d-slice of out = headout @ Wo^T + bo (column-sharded Wo); quarter q's
    matmuls start as soon as its two gathers have landed, so only the last
    quarter's gather latency is exposed.
Host: slices/packs weights per core (bf16), transposes x (bf16), concatenates
disjoint outputs.
"""

import contextlib
import ctypes
import sys
import types

import numpy as np

if "/opt/trn_rl_repo" not in sys.path:
    sys.path.insert(0, "/opt/trn_rl_repo")

# ---------------------------------------------------------------- shims ----


def _install_antenv_shim():
    """Provide antenv.axon_hooks (NTFF profile hook) if the image lacks it."""
    try:
        import antenv.axon_hooks  # noqa: F401

        return
    except ImportError:
        pass

    def _hook_factory():
        so_path = "/opt/axon/libaxon_pjrt.so"
        try:
            lib = ctypes.CDLL(so_path)
        except OSError:
            return None
        if not hasattr(lib, "axon_start_nrt_profile"):
            return None
        lib.axon_start_nrt_profile.argtypes = [
            ctypes.POINTER(ctypes.c_int64),
            ctypes.c_size_t,
        ]
        lib.axon_start_nrt_profile.restype = ctypes.c_int64
        lib.axon_stop_nrt_profile.argtypes = [ctypes.c_char_p]
        lib.axon_stop_nrt_profile.restype = ctypes.c_int64

        @contextlib.contextmanager
        def _hook(output_dir, device_ids):
            import jax

            jax.devices()
            if device_ids:
                ids = (ctypes.c_int64 * len(device_ids))(*device_ids)
                rc = lib.axon_start_nrt_profile(ids, len(device_ids))
            else:
                rc = lib.axon_start_nrt_profile(None, 0)
            if rc != 0:
                raise RuntimeError(f"axon_start_nrt_profile rc={rc}")
            try:
                yield
            finally:
                n = lib.axon_stop_nrt_profile(str(output_dir).encode())
                print(f"ntff profile: {n} file(s) -> {output_dir}", file=sys.stderr)

        return _hook

    hook = _hook_factory()
    mod = types.ModuleType("antenv.axon_hooks")
    mod.get_axon_ntff_profile_hook = lambda: hook
    mod.set_axon_ntff_profile_hook = lambda h: None
    sys.modules["antenv.axon_hooks"] = mod


def _install_tile_drain_patch():
    """This walrus build rejects >1 sync wait on the Tile tail Drain; split the
    waits across chained single-wait drains."""
    import concourse.tile as tile

    if getattr(tile.TileContext, "_drain_patch_installed", False):
        return

    def _drain_and_barrier(self, tick_clock, wait_clock):
        nc = self.nc
        drain_inst = nc.sync.drain()
        wait_clock.add_sem_waits(
            drain_inst.ins, tile.ScopedClock({None: tick_clock.global_clock})
        )
        si = drain_inst.ins.sync_info
        waits = list(si.on_wait) if si is not None and si.on_wait else []
        if len(waits) > 1:
            si.on_wait = waits[:1]
            assert self.sems is not None
            by_num = {h.num: h for h in self.sems.allocated().values()}
            for w in waits[1:]:
                d2 = nc.sync.drain()
                h = by_num.get(w.id)
                assert h is not None, f"no sem handle for wait {w.ant_name}"
                d2.wait_op(h, w.wait_value, "sem-ge", check=False)
        nc.all_engine_barrier()
        assert self.sems is not None
        popped = nc._tile_sem_poison_stack.pop()
        assert popped is self._sem_poison
        nc.clear_and_free_semaphores(list(self.sems.allocated().values()))
        nc.all_engine_barrier()

    tile.TileContext._drain_and_barrier = _drain_and_barrier
    tile.TileContext._drain_patch_installed = True


_install_antenv_shim()


def _split_multi_waits(nc, max_waits=1, pe_zero=False):
    """This walrus build rejects instructions carrying more than ~1 sync wait.
    Move excess waits onto same-engine NOPs inserted immediately before the
    instruction (sequencer waits execute in stream order, so this is
    semantics-preserving). With pe_zero, PE matmuls shed ALL waits onto NOPs
    so the wait-check never serializes against the matmul pipeline drain."""
    import bass_rust
    import concourse.mybir as mybir

    n = 0
    for bb in nc.m.functions[0].blocks:
        insts = bb.instructions
        out = []
        for inst in insts:
            si = inst.sync_info
            waits = list(si.on_wait) if si is not None and si.on_wait else []
            lim = max_waits
            if (
                pe_zero
                and inst.engine == mybir.EngineType.PE
                and isinstance(inst, mybir.InstMatmult)
            ):
                lim = 0
            if len(waits) > lim:
                keep = waits[-lim:] if lim else []
                for w in waits[: len(waits) - lim]:
                    nop = mybir.InstNoOp(name=f"waitnop_{n}", ins=[], outs=[])
                    n += 1
                    nop.engine = inst.engine
                    nop.sync_info = bass_rust.SyncInfo(on_wait=[w], on_update=[])
                    out.append(nop)
                si.on_wait = keep
            out.append(inst)
        if len(out) != len(insts):
            insts[:] = out
    return n


# ------------------------------------------------------------- program -----

N_CORES = 8
GROUP = 4  # cores per batch group

# Schraudolph exp-as-int16-bits: bf16_bits(exp(x)) ~= round(EXP_A*x + EXP_B).
# Calibrated for max rel err ~3.3% over x in [-4, 4]; softmax-ratio errors
# largely cancel in the attention-weighted average.
EXP_A = 128.0 / 0.6931471805599453
EXP_B = 16250.40
# exp split: even heads go to ACT; odd heads to DVE except these tb -> ACT.
# DVE also carries the reciprocal chain + normalize multiplies, so it gets
# fewer exp tiles (19 ACT / 13 DVE balances the two engines' phase-A load).
ACT_ODD_TB = (3, 7, 11)
# magic-number bf16 reciprocal seed (one Newton step follows); host negates
# Wq/bq so the seed's sign flip cancels in the final multiply.
RECIP_MAGIC = 0x7EF4  # seed bits = (den_bits - MAGIC) * -1

last_results = None  # BassKernelResults of the most recent run (for test.py)


def build_program(S=2048, DM=1024, H=16, DK=64, split_waits=True):
    """Emit the SPMD Bass/Tile program. Returns nc."""
    import concourse.bass as bass
    import concourse.mybir as mybir
    import concourse.tile as tile

    _install_tile_drain_patch()

    f32 = mybir.dt.float32
    bf16 = mybir.dt.bfloat16
    i16 = mybir.dt.int16
    NPAIR = 2  # head pairs per core (4 heads)
    NH = 2 * NPAIR  # heads per core
    KT = DM // 128  # contraction chunks for projections
    TT = S // 128  # t tiles (scores row blocks / AV contraction tiles)
    PB = 512  # proj s-block width
    NPB = S // PB
    HDK = H * DK  # concat dim (1024)
    KO = HDK // 128  # outproj contraction chunks
    DSL = HDK // GROUP  # out d-slice per core (256)

    nc = bass.Bass(
        trn_type="TRN2", target_bir_lowering=False, debug=False, num_devices=N_CORES
    )

    def din(name, shape, dt=bf16):
        return nc.dram_tensor(name, shape, dt, kind="ExternalInput").ap()

    # x[b].T per kind, block-major chunk-packed [p, blk, kc, s'] =
    # xT[kc*128+p, blk*PB+s'] so one tile DMA is 4KB-contiguous per partition
    xT = {p: din(f"x{p}T", [128, NPB, KT, PB]) for p in ("q", "k", "v")}
    W = {p: din(f"w{p}", [NPAIR, 128, KT, 128]) for p in ("k", "v")}
    wq4 = din("wq4", [128, KT, NH * DK])  # all 4 heads' Wq.T, chunk-packed
    bq4 = din("bq4", [1, NH * DK])  # bq of the 4 heads (folded into q-proj)
    bk8 = din("bk8", [NPAIR, 128, 1], f32)  # bk / sqrt(dk)
    bv = din("bv", [NPAIR, 128, 1], f32)
    woT = din("woT", [128, KO, DSL])  # Wo.T columns, chunk-packed
    boT = din("boT", [128, 2], f32)  # bo d-slice as [128, 2]
    out_ap = nc.dram_tensor("out", [DSL, S], bf16, kind="ExternalOutput").ap()

    Exp = mybir.ActivationFunctionType.Exp
    mult = mybir.AluOpType.mult
    add = mybir.AluOpType.add

    with tile.TileContext(nc) as tc:
        with contextlib.ExitStack() as ctx:
            sb = ctx.enter_context(tc.tile_pool(name="sb", bufs=2))
            big = ctx.enter_context(tc.tile_pool(name="big", bufs=8))
            ps = ctx.enter_context(tc.tile_pool(name="ps", bufs=2, space="PSUM"))
            dram = ctx.enter_context(tc.tile_pool(name="dram", bufs=1, space="DRAM"))

            # --- startup DMAs in consumption order: the first k-proj matmul
            # waits only on wk + the first xk tile ---
            wsb = {}
            for kind in ("k", "v"):
                wsb[kind] = [
                    sb.tile(
                        [128, KT, 128], bf16, tag="w", bufs=4, name=f"w_{kind}{p}"
                    )
                    for p in range(NPAIR)
                ]
            for p in range(NPAIR):
                nc.sync.dma_start(wsb["k"][p][:, :, :], W["k"][p])

            def dma_xt(kind, blk):
                # four 2-chunk DMAs: the first contraction matmuls wait on a
                # quarter of the tile, and the pieces spread across queues
                t = sb.tile([128, KT, PB], bf16, tag="xt", bufs=3, name=f"xt_{kind}")
                for q in range(0, KT, 2):
                    nc.sync.dma_start(
                        t[:, q : q + 2, :], xT[kind][:, blk, q : q + 2, :]
                    )
                return t

            xt0 = {"k": dma_xt("k", 0)}
            for p in range(NPAIR):
                nc.sync.dma_start(wsb["v"][p][:, :, :], W["v"][p])
            xt0["v"] = dma_xt("v", 0)
            bk_sb = sb.tile([128, NPAIR], f32, tag="bk", bufs=1)
            bv_sb = sb.tile([128, NPAIR], f32, tag="bv", bufs=1)
            for p in range(NPAIR):
                nc.sync.dma_start(bk_sb[:, p : p + 1], bk8[p])
                nc.sync.dma_start(bv_sb[:, p : p + 1], bv[p])
            wq_sb = sb.tile([128, KT, NH * DK], bf16, tag="wq", bufs=1)
            nc.sync.dma_start(wq_sb[:, :, :], wq4[:, :, :])
            ones1r = sb.tile([1, 128], bf16, tag="o1r", bufs=1)
            nc.gpsimd.memset(ones1r[:], 1.0)
            bq4_sb = sb.tile([1, NH * DK], bf16, tag="bq4", bufs=1)
            nc.sync.dma_start(bq4_sb[:], bq4[:])

            # warmup collective: the first collective on the CC stream pays
            # ~11us of one-time setup and runs at ~1/5 bandwidth; pay that
            # during the projection phase when the CC engines are idle.
            wu_in = dram.tile([128, 8], bf16, name="wu_in")
            wu_out = dram.tile([GROUP * 128, 8], bf16, name="wu_out")
            nc.gpsimd.collective_compute(
                "AllGather",
                mybir.AluOpType.bypass,
                replica_groups=[[0, 1, 2, 3], [4, 5, 6, 7]],
                ins=[wu_in.opt()],
                outs=[wu_out.opt()],
            )

            # --- persistent big tiles ---
            k2T = [
                big.tile([128, S], bf16, tag="kv", bufs=4, name=f"k2T_{p}")
                for p in range(NPAIR)
            ]
            v2T = [
                big.tile([128, S], bf16, tag="kv", bufs=4, name=f"v2T_{p}")
                for p in range(NPAIR)
            ]
            # ones-augmented q2 per head: [t, dk|1] chunks of 65 columns per tile
            qa = [
                big.tile([128, TT * 65], bf16, tag="qa", bufs=NH, name=f"qa_{h}")
                for h in range(NH)
            ]
            for h in range(NH):
                nc.gpsimd.memset(qa[h][:], 1.0)
            headout = [
                big.tile([128, S], bf16, tag="ho", bufs=NPAIR, name=f"ho_{p}")
                for p in range(NPAIR)
            ]

            # --- phase P: projections ---
            # k/v: out[dk-pair, s] — stationary W chunk, moving xT chunk.
            # q: out[t, dk-heads] — stationary xT chunk, moving Wq of 4 heads.
            for blk in range(NPB):
                s0 = blk * PB
                for kind in ("k", "v"):
                    pv = [
                        ps.tile([128, PB], f32, tag="av", bufs=4, name=f"pv{p}")
                        for p in range(NPAIR)
                    ]
                    xt = xt0[kind] if blk == 0 else dma_xt(kind, blk)
                    for kc in range(KT):
                        for p in range(NPAIR):
                            nc.tensor.matmul(
                                pv[p][:],
                                wsb[kind][p][:, kc, :],
                                xt[:, kc, :],
                                start=(kc == 0),
                                stop=(kc == KT - 1),
                            )
                    for p in range(NPAIR):
                        if kind == "k":
                            nc.vector.tensor_scalar(
                                k2T[p][:, s0 : s0 + PB],
                                pv[p][:],
                                1.0 / 8.0,
                                bk_sb[:, p : p + 1],
                                mult,
                                add,
                            )
                        else:
                            nc.vector.tensor_scalar_add(
                                v2T[p][:, s0 : s0 + PB], pv[p][:], bv_sb[:, p : p + 1]
                            )
                # one accumulation region per PSUM bank: a start=True matmul
                # clears its whole bank, so each 256-wide q region gets its
                # own one-bank tile (upper 256 columns unused).
                pq = [
                    ps.tile(
                        [128, PB],
                        f32,
                        tag=("sc" if c < 2 else "av"),
                        bufs=4,
                        name=f"pq{c}",
                    )
                    for c in range(4)
                ]
                xtq = dma_xt("q", blk)
                for kc in range(KT):
                    for c in range(4):  # t-chunks inside this s-block
                        nc.tensor.matmul(
                            pq[c][:, 0:256],
                            xtq[:, kc, c * 128 : (c + 1) * 128],
                            wq_sb[:, kc, :],
                            start=(kc == 0),
                            stop=False,
                        )
                for c in range(4):  # fold bq in: pq += ones^T (x) bq4
                    nc.tensor.matmul(
                        pq[c][:, 0:256],
                        ones1r[:],
                        bq4_sb[:],
                        start=False,
                        stop=True,
                    )
                for c in range(4):
                    tcix = blk * 4 + c
                    for h in range(NH):
                        nc.vector.tensor_copy(
                            qa[h][:, tcix * 65 : tcix * 65 + 64],
                            pq[c][:, h * 64 : h * 64 + 64],
                        )

            boT_sb = sb.tile([128, 2], f32, tag="bo", bufs=1)
            nc.sync.dma_start(boT_sb[:], boT[:])
            woT_sb = sb.tile([128, KO, DSL], bf16, tag="wo", bufs=1)
            nc.sync.dma_start(woT_sb[:, :, :], woT[:, :, :])

            # --- phase A: attention over (pair, s-quarter) blocks, pair-
            # alternating so the per-(pair, s-qtr) gathers spread evenly ---
            # scores for the pair's two heads run as row-split tile_position
            # matmuls (K=64 halves of the PE array, concurrent); exp split
            # ACT/DVE; AV per head with ones-augmented q2aug; normalize is a
            # 3-stage pipeline (recip at +1 block, multiply+gather at +2) so
            # no engine waits on the broadcast DMA round trip.
            SQA = 512
            NSHA = S // SQA
            blocks = [(p, sh) for sh in range(NSHA) for p in range(NPAIR)]
            NB = len(blocks)
            expt = [
                [
                    [
                        big.tile(
                            [128, SQA],
                            i16,
                            tag="expt",
                            bufs=4 * TT,
                            name=f"e{par}_{hh}_{tb}",
                        )
                        for tb in range(TT)
                    ]
                    for hh in range(2)
                ]
                for par in range(2)
            ]
            av_tiles = {}
            rec_bufs = {}
            cc_in = [
                [dram.tile([128, SQA], bf16, name=f"cc_in_{p}_{sh}") for sh in range(NSHA)]
                for p in range(NPAIR)
            ]
            cc_out = [
                [
                    dram.tile([GROUP * 128, SQA], bf16, name=f"cc_out_{p}_{sh}")
                    for sh in range(NSHA)
                ]
                for p in range(NPAIR)
            ]

            def av_step(i, step):
                # contract t-chunks in REVERSE production order: the first AV
                # of the block waits on the newest exp tile (highest sem
                # value), so every later AV's wait is dominated and elided --
                # wait-free PE instructions pipeline into the previous
                # matmul's SBUF-access drain window instead of paying it.
                p, sh = blocks[i]
                tk = TT - 1 - step
                if step == 0:
                    av_tiles[i] = [
                        ps.tile([65, SQA], f32, tag="av", bufs=4, name=f"av{hh}")
                        for hh in range(2)
                    ]
                eb = expt[i % 2]
                for hh in range(2):
                    nc.tensor.matmul(
                        av_tiles[i][hh][:],
                        qa[2 * p + hh][:, tk * 65 : tk * 65 + 65],
                        eb[hh][tk][:].bitcast(bf16),
                        start=(step == 0),
                        stop=(step == TT - 1),
                    )

            def emit_recip(i, hh):
                # stage 1: reciprocal of the denominator row (magic bf16 seed
                # + one Newton step; sign flip cancels against the host-
                # negated Wq/bq) + DMA broadcast to 64 partitions; consumed
                # by emit_mul one block later. Per-head granularity keeps
                # DVE bursts short so exp service never lags far.
                av = av_tiles[i][hh]
                # bf16 bits of the f32 denominator = its high bytes, via
                # a stride-2 int16 view straight out of PSUM
                r0 = sb.tile([1, SQA], i16, tag="r0", bufs=4, name="r0")
                nc.vector.tensor_scalar(
                    r0[:],
                    av[64:65, :].bitcast(i16)[:, 1::2],
                    RECIP_MAGIC,
                    -1,
                    mybir.AluOpType.subtract,
                    mult,
                )
                t1 = sb.tile([1, SQA], f32, tag="t1", bufs=4, name="t1")
                nc.vector.tensor_mul(t1[:], av[64:65, :], r0[:].bitcast(bf16))
                rec = sb.tile([1, SQA], f32, tag="rec", bufs=4, name="rec")
                nc.vector.scalar_tensor_tensor(
                    rec[:],
                    t1[:],
                    2.0,
                    r0[:].bitcast(bf16),
                    mybir.AluOpType.subtract,
                    mult,
                )
                rdr = dram.tile([1, SQA], f32, name=f"rdr_{i}_{hh}")
                nc.sync.dma_start(rdr[:], rec[:])
                bcb = sb.tile([64, SQA], f32, tag="bcb", bufs=4, name="bcb")
                s2b, _ = bass.broadcast_tensor_aps(rdr[:], bcb[:])
                nc.sync.dma_start(bcb[:], s2b)
                rec_bufs.setdefault(i, []).append(bcb)

            def emit_mul(i, hh):
                # stage 2: normalize multiply + stage headout for one head
                p, sh = blocks[i]
                prow = 64 * hh
                av = av_tiles[i][hh]
                dst = headout[p][prow : prow + 64, sh * SQA : (sh + 1) * SQA]
                nc.vector.tensor_mul(dst, av[0:64, :], rec_bufs[i][hh][:])
                nc.sync.dma_start(cc_in[p][sh][prow : prow + 64, :], dst)

            def emit_cc(i):
                p, sh = blocks[i]
                nc.gpsimd.collective_compute(
                    "AllGather",
                    mybir.AluOpType.bypass,
                    replica_groups=[[0, 1, 2, 3], [4, 5, 6, 7]],
                    ins=[cc_in[p][sh].opt()],
                    outs=[cc_out[p][sh].opt()],
                )
                del av_tiles[i]
                del rec_bufs[i]

            for j in range(NB):
                p, sh = blocks[j]
                eb = expt[j % 2]
                for tb in range(TT):
                    for hh in range(2):
                        sc = ps.tile(
                            [128, SQA], f32, tag="sc", bufs=4, name=f"sc{hh}"
                        )
                        nc.tensor.matmul(
                            sc[:],
                            k2T[p][64 * hh : 64 * hh + 64, tb * 128 : (tb + 1) * 128],
                            v2T[p][64 * hh : 64 * hh + 64, sh * SQA : (sh + 1) * SQA],
                            start=True,
                            stop=True,
                            tile_position=(64 * hh, 0),
                        )
                        if hh == 0 or tb in ACT_ODD_TB:
                            nc.scalar.activation(
                                eb[hh][tb][:].bitcast(bf16), sc[:], Exp
                            )
                        else:
                            nc.vector.tensor_scalar(
                                eb[hh][tb][:], sc[:], EXP_A, EXP_B, mult, add
                            )
                    if j > 0:
                        av_step(j - 1, tb)
                    if tb == 6 and j > 1:
                        # mid-block so the DVE multiply lands after the bcb
                        # broadcast round trip and the av PSUM ring frees a
                        # full block before the next allocation needs it
                        emit_mul(j - 2, 0)
                        emit_mul(j - 2, 1)
                        emit_cc(j - 2)
                if j > 0:
                    emit_recip(j - 1, 0)
                    emit_recip(j - 1, 1)
            for tk in range(TT):
                av_step(NB - 1, tk)
                if tk == 2:
                    # fire block NB-2's gather while the last AVs still run
                    emit_mul(NB - 2, 0)
                    emit_mul(NB - 2, 1)
                    emit_cc(NB - 2)
            emit_recip(NB - 1, 0)
            emit_recip(NB - 1, 1)
            emit_mul(NB - 1, 0)
            emit_mul(NB - 1, 1)
            emit_cc(NB - 1)

            # --- phase O: output projection per s-quarter, transposed layout
            # outT[d, s]. Quarter q depends only on gathers (0, q) and (1, q);
            # quarters 0-2 are ready well before the attention tail, so only
            # the last quarter's gather latency is exposed. ---
            OSW = SQA
            for sblk in range(NSHA):
                pos = [
                    ps.tile([128, OSW], f32, tag="sc", bufs=4, name=f"po{dblk}")
                    for dblk in range(2)
                ]
                for ci, (p, g) in enumerate(
                    [(p, g) for p in range(NPAIR) for g in range(GROUP)]
                ):
                    chh = sb.tile([128, OSW], bf16, tag="ch", bufs=4, name="chh")
                    nc.sync.dma_start(
                        chh[:], cc_out[p][sblk][128 * g : 128 * g + 128, :]
                    )
                    for dblk in range(2):
                        nc.tensor.matmul(
                            pos[dblk][:],
                            woT_sb[:, 2 * g + p, 128 * dblk : 128 * (dblk + 1)],
                            chh[:],
                            start=(ci == 0),
                            stop=(ci == KO - 1),
                        )
                for dblk in range(2):
                    ob = sb.tile([128, OSW], bf16, tag="ob", bufs=3, name="ob")
                    nc.vector.tensor_scalar_add(
                        ob[:], pos[dblk][:], boT_sb[:, dblk : dblk + 1]
                    )
                    nc.sync.dma_start(
                        out_ap[
                            128 * dblk : 128 * (dblk + 1),
                            sblk * OSW : (sblk + 1) * OSW,
                        ],
                        ob[:],
                    )

    if split_waits:
        _split_multi_waits(nc)
    return nc


def make_in_maps(v, k, q, Wq, bqv, Wk, bkv, Wv, bvv, Wo, bov, S, DM, H, DK):
    """Per-core input dicts from full inputs (host-side prep is slicing /
    transpose / dtype conversion)."""
    import ml_dtypes

    bfdt = ml_dtypes.bfloat16
    HDK = H * DK
    DSL = HDK // GROUP
    KT = DM // 128
    PB = 512
    NPB = S // PB

    def chunkpack(a):  # [DM, N] -> [128, KT, N]
        return np.ascontiguousarray(
            a.reshape(KT, 128, a.shape[1]).transpose(1, 0, 2)
        ).astype(bfdt)

    def blockpack(a):  # [DM, S] -> [128, NPB, KT, PB] (block-major)
        return np.ascontiguousarray(
            a.reshape(KT, 128, NPB, PB).transpose(1, 2, 0, 3)
        ).astype(bfdt)

    xT = {}
    for b in range(2):
        xT[("q", b)] = blockpack(q[b].T)
        xT[("k", b)] = blockpack(k[b].T)
        xT[("v", b)] = blockpack(v[b].T)
    WoT = np.ascontiguousarray(Wo.T)  # [HDK, HDK_out]
    in_maps = []
    for c in range(N_CORES):
        b = c // GROUP
        h0 = 4 * (c % GROUP)
        m = {
            "xqT": xT[("q", b)],
            "xkT": xT[("k", b)],
            "xvT": xT[("v", b)],
        }
        for kind, Wt, bt in (("k", Wk, bkv), ("v", Wv, bvv)):
            wp = np.empty((2, DM, 128), np.float32)
            bp = np.empty((2, 128, 1), np.float32)
            for p in range(2):
                ha, hb = h0 + 2 * p, h0 + 2 * p + 1
                wp[p, :, :64] = Wt[ha].T
                wp[p, :, 64:] = Wt[hb].T
                bp[p, :64, 0] = bt[ha]
                bp[p, 64:, 0] = bt[hb]
            m[f"w{kind}"] = np.ascontiguousarray(
                wp.reshape(2, KT, 128, 128).transpose(0, 2, 1, 3)
            ).astype(bfdt)
            if kind == "k":
                m["bk8"] = bp / 8.0
            else:
                m["bv"] = bp
        m["wq4"] = chunkpack(
            -np.concatenate([Wq[h0 + h].T for h in range(4)], axis=1)
        )
        m["bq4"] = (
            -np.concatenate([bqv[h0 + h] for h in range(4)])
            .reshape(1, -1)
            .astype(bfdt)
        )
        d0 = DSL * (c % GROUP)
        m["woT"] = chunkpack(WoT[:, d0 : d0 + DSL])
        m["boT"] = np.ascontiguousarray(bov[d0 : d0 + DSL].reshape(2, 128).T)
        in_maps.append(m)
    return in_maps


def kernel(v, k, q, Wq, bq, Wk, bk, Wv, bv, Wo, bo, _trace=False):
    """Full inputs in, full output out. Runs the SPMD Bass kernel on 8 cores."""
    global last_results
    from concourse.bass_utils import run_bass_kernel_spmd

    v, k, q = (np.asarray(a, np.float32) for a in (v, k, q))
    B, S, DM = q.shape
    H, DK = Wq.shape[0], Wq.shape[1]
    HDK = H * DK
    DSL = HDK // GROUP

    nc = build_program(S=S, DM=DM, H=H, DK=DK)
    in_maps = make_in_maps(
        v,
        k,
        q,
        *(np.asarray(a, np.float32) for a in (Wq, bq, Wk, bk, Wv, bv, Wo, bo)),
        S=S,
        DM=DM,
        H=H,
        DK=DK,
    )
    res = run_bass_kernel_spmd(nc, in_maps, list(range(N_CORES)), trace=_trace)
    last_results = res
    out = np.empty((B, S, HDK), np.float32)
    for c in range(N_CORES):
        b = c // GROUP
        d0 = DSL * (c % GROUP)
        out[b, :, d0 : d0 + DSL] = res.results[c]["out"].astype(np.float32).T
    return out
